# revision 10
# baseline (speedup 1.0000x reference)
# Trainium2 Bass kernel for nn_DecoderLayer_17910013625017 (sparse_attention).
#
# Strategy (8 NeuronCores, two SPMD launches, host combines partials):
#   Launch 1 (attention): tensor-parallel over heads. Core i owns q-heads
#     {2i, 2i+1} and kv-head i (GQA groups stay core-local). Each core
#     computes qkv projections, q/k rms-norm + rope, causal sparse attention
#     (mask derived from keys_idxs/hs_idxs compares on-device; blocks above
#     the causal band are skipped), the o-projection row-shard, and its
#     2-head share of the token-importance softmax. Output: a [2048, 2048]
#     partial of attn_out in transposed ([out_dim, seq]) layout.
#   Host: hT = hidden^T + sum(partials)  (the residual add + TP-reduce).
#   Launch 2 (MLP): tensor-parallel over the FFN dim (768 per core).
#     Output: a [2048, 2048] partial of the MLP in natural layout.
#   Host: out = hT^T + sum(mlp partials); importance from summed per-core
#     softmax shares with imp[-1] = inf.
#
# All matmuls run as float32r (full fp32 bits in memory; the PE streams at
# 1 cycle/row for N>=256). Index and rope-angle data never passes through
# f32r matmuls: those broadcasts use exact DMA replication. Softmax skips
# max-subtraction (|scores| <= 11.32 by Cauchy-Schwarz since q/k rows are
# rms-normalized).

import os
import sys
import time

for _p in ("/opt/trn_rl_repo",):
    if _p not in sys.path:
        sys.path.insert(0, _p)

import numpy as np

import jax

try:
    jax.config.update("jax_compilation_cache_dir", os.environ.get(
        "KERNEL_JAX_CACHE", "/tmp/decoder_kernel_jaxcache"))
    jax.config.update("jax_persistent_cache_min_entry_size_bytes", 0)
    jax.config.update("jax_persistent_cache_min_compile_time_secs", 0.0)
except Exception:
    pass

from contextlib import ExitStack

import concourse.bass as bass
import concourse.mybir as mybir
import concourse.tile as tile
from concourse import bacc
from concourse.bass import RegisterHandles, make_scalar_value
from concourse.bass_utils import run_bass_kernel_spmd
from concourse.masks import make_identity

F32 = mybir.dt.float32
F32R = mybir.dt.float32r
I32 = mybir.dt.int32
AF = mybir.ActivationFunctionType
ALU = mybir.AluOpType

S = 2048          # sequence length (kept tokens)
H = 2048          # hidden dim
HD = 128          # head dim
NH = 16           # query heads
FF = 6144         # ffn dim
NCORES = 8
QH = NH // NCORES           # 2 q-heads per core
FFS = FF // NCORES          # 768 ffn per core
EPS = 1e-6
SCALING = float(HD ** -0.5)

TWO_PI = 2.0 * np.pi
CW_C1 = 6.28125
_c2i = np.float32(TWO_PI - CW_C1).view(np.uint32) & np.uint32(0xFFFFF000)
CW_C2 = float(_c2i.view(np.float32))
CW_C3 = float(np.float32(TWO_PI - CW_C1 - CW_C2))
INV2PI = float(np.float32(1.0 / TWO_PI))
MAGIC = 8388608.0  # 2^23: x + MAGIC - MAGIC == round(x) for 0 <= x < 2^22

NSL = 4            # seq slices of 512
SL = S // NSL      # 512
NKB = S // HD      # 16 key blocks of 128
NCH = H // 128     # 16 contraction chunks


def _all_engine_value(nc, ap, name, max_val):
    regs = []
    for e in mybir.ALL_ENGINES:
        eng = nc.engines[e]
        r = eng.alloc_register(f"{name}_{e.name}")
        eng.reg_load(r, ap)
        regs.append(r)
    return make_scalar_value(RegisterHandles(regs), min_val=0, max_val=max_val)


def build_l1():
    nc = bacc.Bacc("TRN2", target_bir_lowering=False, debug=False)

    hT_d = nc.dram_tensor("hT", [H, S], F32, kind="ExternalInput").ap()
    wq_d = nc.dram_tensor("wq", [H, QH * HD], F32, kind="ExternalInput").ap()
    wk_d = nc.dram_tensor("wk", [H, HD], F32, kind="ExternalInput").ap()
    wv_d = nc.dram_tensor("wv", [H, HD], F32, kind="ExternalInput").ap()
    wo_d = nc.dram_tensor("wo", [QH * HD, H], F32, kind="ExternalInput").ap()
    pos_d = nc.dram_tensor("pos", [1, S], F32, kind="ExternalInput").ap()
    hs1_d = nc.dram_tensor("hs1", [1, S], F32, kind="ExternalInput").ap()
    keys_d = nc.dram_tensor("keys", [S], F32, kind="ExternalInput").ap()
    lnin_d = nc.dram_tensor("lnin", [H], F32, kind="ExternalInput").ap()
    invf_d = nc.dram_tensor("invf", [HD, 1], F32, kind="ExternalInput").ap()
    qnw_d = nc.dram_tensor("qnw", [1, HD], F32, kind="ExternalInput").ap()
    knw_d = nc.dram_tensor("knw", [1, HD], F32, kind="ExternalInput").ap()
    ones_d = nc.dram_tensor("ones", [128, 128], F32, kind="ExternalInput").ap()
    nit_d = nc.dram_tensor("nit", [1, 1], I32, kind="ExternalInput").ap()

    ap_d = nc.dram_tensor("attn_part", [H, S], F32, kind="ExternalOutput").ap()
    imp_d = nc.dram_tensor("imp_part", [1, S], F32, kind="ExternalOutput").ap()
    s_dram = nc.dram_tensor("s_scratch", [1, S], F32).ap()

    with tile.TileContext(nc) as tc, ExitStack() as top:
        cpool = top.enter_context(tc.tile_pool(name="const0", bufs=1))
        nit_sb = cpool.tile([1, 1], I32)
        nc.sync.dma_start(nit_sb[:], nit_d[:])
        nv = _all_engine_value(nc, nit_sb[0:1, 0:1], "nit", 1 << 20)

        with tc.For_i(0, nv, 1, hint_engines=tuple(mybir.ALL_ENGINES)):
            with ExitStack() as it:
                const = it.enter_context(tc.tile_pool(name="c1", bufs=1))
                onesr = const.tile([128, 128], F32R)
                nc.sync.dma_start(onesr[:], ones_d[:].bitcast(F32R))
                ident = const.tile([128, 128], F32)
                make_identity(nc, ident)
                qnw = const.tile([1, HD], F32R)
                nc.sync.dma_start(qnw[:], qnw_d[:].bitcast(F32R))
                knw = const.tile([1, HD], F32R)
                nc.sync.dma_start(knw[:], knw_d[:].bitcast(F32R))
                invf = const.tile([HD, 1], F32)
                nc.sync.dma_start(invf[:], invf_d[:])
                keys_sb = const.tile([128, NKB], F32)
                nc.sync.dma_start(keys_sb[:], keys_d.rearrange("(b p) -> p b", p=128))
                lnin_sb = const.tile([128, NCH], F32)
                nc.sync.dma_start(lnin_sb[:], lnin_d.rearrange("(c p) -> p c", p=128))
                s_col = const.tile([128, NKB], F32)

                srow = it.enter_context(tc.tile_pool(name="srow", bufs=1))
                s_row = srow.tile([1, S], F32, tag="s_row")

                qk_res = it.enter_context(tc.tile_pool(name="qkres", bufs=1))
                qT0r = qk_res.tile([128, S], F32, tag="qT0r")
                qT1r = qk_res.tile([128, S], F32, tag="qT1r")
                kTr = qk_res.tile([128, S], F32, tag="kTr")
                vTr = qk_res.tile([128, S], F32, tag="vTr")

                # ---- Phase A: qkv^T projections + hidden sum-of-squares.
                with ExitStack() as ph:
                    wpool = ph.enter_context(tc.tile_pool(name="wqkv", bufs=1))
                    wq_t = wpool.tile([128, NCH, QH * HD], F32R)
                    nc.sync.dma_start(
                        wq_t[:], wq_d.rearrange("(c p) n -> p c n", p=128).bitcast(F32R))
                    wk_t = wpool.tile([128, NCH, HD], F32R)
                    nc.sync.dma_start(
                        wk_t[:], wk_d.rearrange("(c p) n -> p c n", p=128).bitcast(F32R))
                    wv_t = wpool.tile([128, NCH, HD], F32R)
                    nc.sync.dma_start(
                        wv_t[:], wv_d.rearrange("(c p) n -> p c n", p=128).bitcast(F32R))
                    for c in range(NCH):
                        sc = lnin_sb[:, c:c + 1]
                        nc.vector.tensor_scalar(wq_t[:, c, :], wq_t[:, c, :].bitcast(F32),
                                                sc, None, op0=ALU.mult)
                        nc.vector.tensor_scalar(wk_t[:, c, :], wk_t[:, c, :].bitcast(F32),
                                                sc, None, op0=ALU.mult)
                        nc.vector.tensor_scalar(wv_t[:, c, :], wv_t[:, c, :].bitcast(F32),
                                                sc, None, op0=ALU.mult)

                    arow = ph.enter_context(tc.tile_pool(name="arow", bufs=1))
                    ssq = arow.tile([1, S], F32, tag="ssq")
                    rtmp = arow.tile([1, S], F32, tag="rtmp")

                    hpool = ph.enter_context(tc.tile_pool(name="ht1", bufs=6))
                    sqp = ph.enter_context(tc.tile_pool(name="sq1", bufs=3))
                    psA = ph.enter_context(tc.tile_pool(name="psA", bufs=6, space="PSUM"))
                    psS = ph.enter_context(tc.tile_pool(name="psS", bufs=2, space="PSUM"))
                    for ss in range(NSL):
                        ps_ss = psS.tile([1, SL], F32, tag="ssq_ps")
                        pq0 = psA.tile([128, SL], F32, tag="acc", name="pq0")
                        pq1 = psA.tile([128, SL], F32, tag="acc", name="pq1")
                        pk = psA.tile([128, SL], F32, tag="acc", name="pk")
                        pv = psA.tile([128, SL], F32, tag="acc", name="pv")
                        for c in range(NCH):
                            ht = hpool.tile([128, SL], F32R)
                            nc.sync.dma_start(
                                ht[:], hT_d[c * 128:(c + 1) * 128,
                                            ss * SL:(ss + 1) * SL].bitcast(F32R))
                            sq = sqp.tile([128, SL], F32R)
                            nc.scalar.activation(sq[:], ht[:].bitcast(F32), AF.Square)
                            st, sp = (c == 0), (c == NCH - 1)
                            nc.tensor.matmul(ps_ss[:], onesr[:, 0:1], sq[:], start=st, stop=sp)
                            nc.tensor.matmul(pq0[:], wq_t[:, c, 0:128], ht[:], start=st, stop=sp)
                            nc.tensor.matmul(pq1[:], wq_t[:, c, 128:256], ht[:], start=st, stop=sp)
                            nc.tensor.matmul(pk[:], wk_t[:, c, :], ht[:], start=st, stop=sp)
                            nc.tensor.matmul(pv[:], wv_t[:, c, :], ht[:], start=st, stop=sp)
                        sl = slice(ss * SL, (ss + 1) * SL)
                        nc.vector.tensor_copy(qT0r[:, sl], pq0[:])
                        nc.vector.tensor_copy(qT1r[:, sl], pq1[:])
                        nc.vector.tensor_copy(kTr[:, sl], pk[:])
                        nc.vector.tensor_copy(vTr[:, sl], pv[:])
                        nc.scalar.copy(ssq[:, sl], ps_ss[:])
                    # s = rsqrt(ssq/H + eps)
                    nc.vector.tensor_scalar(ssq[:], ssq[:], 1.0 / H, EPS,
                                            op0=ALU.mult, op1=ALU.add)
                    nc.vector.reciprocal(rtmp[:], ssq[:])
                    nc.scalar.sqrt(s_row[:], rtmp[:])
                    nc.sync.dma_start(s_dram[:], s_row[:])
                    nc.sync.dma_start(s_col[:], s_dram.rearrange("o (b p) -> p (o b)", p=128))

                # ---- Phase B: rope tables; q/k norm + rope; v scale+transpose.
                qk_fin = it.enter_context(tc.tile_pool(name="qkfin", bufs=1))
                qT = [qk_fin.tile([128, S], F32R, tag=f"qT{h}", name=f"qT{h}") for h in range(QH)]
                kT = qk_fin.tile([128, S], F32R, tag="kT")
                v_res = qk_fin.tile([128, NKB, HD], F32R, tag="vres")
                with ExitStack() as ph:
                    scr = ph.enter_context(tc.tile_pool(name="ropescr", bufs=1))
                    posb = scr.tile([64, S], F32, tag="posb")
                    nc.sync.dma_start(posb[:], pos_d[:].broadcast_to((64, S)))
                    sin_s = scr.tile([128, S], F32, tag="sin_s")
                    cos_s = scr.tile([128, S], F32, tag="cos_s")
                    frq = scr.tile([64, S], F32, tag="t1", name="frq")
                    nc.vector.tensor_scalar(frq[:], posb[:], invf[0:64, 0:1], None, op0=ALU.mult)
                    t5 = scr.tile([64, S], F32, tag="t2", name="t5")
                    nc.vector.tensor_scalar(t5[:], frq[:], INV2PI, MAGIC, op0=ALU.mult, op1=ALU.add)
                    kk = scr.tile([64, S], F32, tag="t3", name="kk")
                    nc.vector.tensor_scalar(kk[:], t5[:], -MAGIC, None, op0=ALU.add)
                    rr = scr.tile([64, S], F32, tag="t2", name="rr")
                    nc.vector.cody_waite_cascade(rr[:], frq[:], kk[:], CW_C1, CW_C2, CW_C3)
                    ys = scr.tile([64, S], F32, tag="t1", name="ys")
                    nc.vector.add_range_wrap(ys[:], rr[:], 0.0, float(np.pi), float(TWO_PI))
                    yc = scr.tile([64, S], F32, tag="t3", name="yc")
                    nc.vector.add_range_wrap(yc[:], rr[:], float(np.pi / 2), float(np.pi), float(TWO_PI))
                    # sin_s rows 0:64 hold -sin (rotate-half sign pre-applied)
                    sin64 = scr.tile([64, S], F32, tag="t2", name="sin64")
                    nc.scalar.activation(sin64[:], ys[:], AF.Sin)
                    nc.scalar.activation(cos_s[0:64, :], yc[:], AF.Sin)
                    nc.vector.tensor_scalar(sin_s[0:64, :], sin64[:], -1.0, None, op0=ALU.mult)
                    nc.vector.tensor_copy(sin_s[64:128, :], sin64[:])
                    nc.vector.tensor_copy(cos_s[64:128, :], cos_s[0:64, :])

                    brow = ph.enter_context(tc.tile_pool(name="brow", bufs=1))
                    s2 = brow.tile([1, S], F32, tag="s2")
                    nc.vector.tensor_mul(s2[:], s_row[:], s_row[:])
                    msq = brow.tile([1, S], F32, tag="msq")
                    grc = brow.tile([1, S], F32, tag="grc")
                    g = brow.tile([1, S], F32R, tag="g")

                    sqp2 = ph.enter_context(tc.tile_pool(name="sq2", bufs=1))
                    psB = ph.enter_context(tc.tile_pool(name="psB", bufs=2, space="PSUM"))
                    psR = ph.enter_context(tc.tile_pool(name="psR", bufs=2, space="PSUM"))
                    gsc = ph.enter_context(tc.tile_pool(name="gscr", bufs=1))
                    heads = [(qT0r, qT[0], qnw, True), (qT1r, qT[1], qnw, True),
                             (kTr, kT, knw, False)]
                    for hi, (raw, fin, nw, is_q) in enumerate(heads):
                        sqh = sqp2.tile([128, S], F32R, tag="sqh")
                        nc.scalar.activation(sqh[:], raw[:], AF.Square)
                        for sl4 in range(NSL):
                            pr = psR.tile([1, SL], F32, tag="msq_ps")
                            nc.tensor.matmul(pr[:], onesr[:, 0:1],
                                             sqh[:, sl4 * SL:(sl4 + 1) * SL], start=True, stop=True)
                            nc.scalar.copy(msq[:, sl4 * SL:(sl4 + 1) * SL], pr[:])
                        # g = s * rsqrt(s^2 * msq/HD + eps) [* HD^-0.5 for q]
                        nc.vector.tensor_mul(msq[:], s2[:], msq[:])
                        nc.vector.tensor_scalar(msq[:], msq[:], 1.0 / HD, EPS,
                                                op0=ALU.mult, op1=ALU.add)
                        nc.vector.reciprocal(grc[:], msq[:])
                        nc.scalar.sqrt(msq[:], grc[:])
                        mul2 = SCALING if is_q else 1.0
                        nc.vector.scalar_tensor_tensor(g[:], msq[:], mul2, s_row[:],
                                                       op0=ALU.mult, op1=ALU.mult)
                        gb = gsc.tile([128, S], F32, tag="gb")
                        for sl4 in range(NSL):
                            pb = psB.tile([128, SL], F32, tag="gb_ps")
                            nc.tensor.matmul(pb[:], nw[:], g[:, sl4 * SL:(sl4 + 1) * SL],
                                             start=True, stop=True)
                            nc.scalar.copy(gb[:, sl4 * SL:(sl4 + 1) * SL], pb[:])
                        # scale in place, then rope into fin
                        nc.vector.tensor_mul(raw[:], raw[:], gb[:])
                        rot = gsc.tile([128, S], F32, tag="roperot")
                        nc.vector.tensor_copy(rot[0:64, :], raw[64:128, :])
                        nc.vector.tensor_copy(rot[64:128, :], raw[0:64, :])
                        nc.vector.tensor_mul(rot[:], rot[:], sin_s[:])
                        nc.vector.tensor_mul(fin[:], raw[:], cos_s[:])
                        nc.vector.tensor_add(fin[:], fin[:].bitcast(F32), rot[:])
                    # v: transpose to [seq, hd] blocks, scaling rows by s
                    psV = ph.enter_context(tc.tile_pool(name="psV", bufs=2, space="PSUM"))
                    for kb in range(NKB):
                        pv2 = psV.tile([128, HD], F32, tag="vtp")
                        nc.tensor.transpose(pv2[:], vTr[:, kb * 128:(kb + 1) * 128], ident[:])
                        nc.vector.tensor_scalar(v_res[:, kb, :], pv2[:], s_col[:, kb:kb + 1],
                                                None, op0=ALU.mult)

                # ---- Phase C: attention (scores^T layout) + importance.
                ctx_res = it.enter_context(tc.tile_pool(name="ctxres", bufs=1))
                ctx = [ctx_res.tile([128, S], F32R, tag=f"ctx{h}", name=f"ctx{h}") for h in range(QH)]
                with ExitStack() as ph:
                    cmask = ph.enter_context(tc.tile_pool(name="cmask", bufs=1))
                    hs1b = cmask.tile([128, S], F32)
                    nc.sync.dma_start(hs1b[:], hs1_d[:].broadcast_to((128, S)))
                    mbp = ph.enter_context(tc.tile_pool(name="mb", bufs=1))
                    expp = ph.enter_context(tc.tile_pool(name="expt", bufs=4))
                    psSC = ph.enter_context(tc.tile_pool(name="psSC", bufs=2, space="PSUM"))
                    psCT = ph.enter_context(tc.tile_pool(name="psCT", bufs=2, space="PSUM"))
                    psSE = ph.enter_context(tc.tile_pool(name="psSE", bufs=2, space="PSUM"))
                    sep = ph.enter_context(tc.tile_pool(name="sep", bufs=2))
                    for ss in range(NSL):
                        qsl = slice(ss * SL, (ss + 1) * SL)
                        # causal band: blocks [4ss, 4ss+4] are masked (ties may
                        # spill one block past the diagonal); blocks < 4ss are
                        # fully visible; blocks > 4ss+4 contribute nothing.
                        first_partial = 4 * ss
                        last_vis = min(4 * ss + 4, NKB - 1)
                        mbs = {}
                        for pb in range(first_partial, last_vis + 1):
                            d = mbp.tile([128, SL], F32, tag=f"mb{pb - first_partial}",
                                         name=f"mb{pb - first_partial}")
                            nc.gpsimd.tensor_scalar(d[:], hs1b[:, qsl], keys_sb[:, pb:pb + 1],
                                                    None, op0=ALU.subtract)
                            nc.gpsimd.tensor_scalar(d[:], d[:], 0.0, 1.0, op0=ALU.max, op1=ALU.min)
                            mbs[pb] = d
                        for h in range(QH):
                            pctx = psCT.tile([128, SL], F32, tag="pctx")
                            pse = psSE.tile([1, SL], F32, tag="pse", name="pse")
                            for kb in range(last_vis + 1):
                                psc = psSC.tile([128, SL], F32, tag="psc", name="psc")
                                nc.tensor.matmul(psc[:], kT[:, kb * 128:(kb + 1) * 128],
                                                 qT[h][:, qsl], start=True, stop=True)
                                ex = expp.tile([128, SL], F32R, tag="ex")
                                nc.scalar.activation(ex[:], psc[:], AF.Exp)
                                if kb in mbs:
                                    nc.vector.tensor_tensor(ex[:], ex[:].bitcast(F32),
                                                            mbs[kb][:], op=ALU.mult)
                                st, sp = (kb == 0), (kb == last_vis)
                                nc.tensor.matmul(pctx[:], v_res[:, kb, :], ex[:], start=st, stop=sp)
                                nc.tensor.matmul(pse[:], onesr[:, 0:1], ex[:], start=st, stop=sp)
                            se = sep.tile([1, SL], F32, tag="se")
                            nc.scalar.copy(se[:], pse[:])
                            rec = sep.tile([1, SL], F32, tag="rec")
                            nc.vector.reciprocal(rec[:], se[:])
                            recr = sep.tile([1, SL], F32R, tag="recr")
                            nc.vector.tensor_copy(recr[:], rec[:])
                            prb = psSC.tile([128, SL], F32, tag="psc", name="prb")
                            nc.tensor.matmul(prb[:], onesr[0:1, :], recr[:], start=True, stop=True)
                            recb = sep.tile([128, SL], F32, tag="recb")
                            nc.scalar.copy(recb[:], prb[:])
                            nc.vector.tensor_tensor(ctx[h][:, qsl], pctx[:], recb[:], op=ALU.mult)
                    # importance from last query (all key blocks, no mask)
                    irows = ph.enter_context(tc.tile_pool(name="irows", bufs=1))
                    ie = irows.tile([1, S], F32, tag="ie")
                    ie2 = irows.tile([1, S], F32, tag="ie2")
                    imp_out = irows.tile([1, S], F32, tag="impout")
                    for h in range(QH):
                        for sl4 in range(NSL):
                            pi = psSE.tile([1, SL], F32, tag="pse", name="pi")
                            nc.tensor.matmul(pi[:], qT[h][:, S - 1:S],
                                             kT[:, sl4 * SL:(sl4 + 1) * SL], start=True, stop=True)
                            nc.scalar.activation(ie[:, sl4 * SL:(sl4 + 1) * SL], pi[:], AF.Exp)
                        isum = irows.tile([1, 1], F32, tag="isum")
                        nc.vector.tensor_reduce(isum[:], ie[:], axis=mybir.AxisListType.X, op=ALU.add)
                        irec = irows.tile([1, 1], F32, tag="irec")
                        nc.vector.reciprocal(irec[:], isum[:])
                        if h == 0:
                            nc.vector.tensor_scalar(imp_out[:], ie[:], irec[0:1, 0:1],
                                                    None, op0=ALU.mult)
                        else:
                            nc.vector.tensor_scalar(ie2[:], ie[:], irec[0:1, 0:1],
                                                    None, op0=ALU.mult)
                            nc.vector.tensor_add(imp_out[:], imp_out[:], ie2[:])
                    nc.sync.dma_start(imp_d[:], imp_out[:])

                # ---- Phase D: o-projection (transposed partial out).
                with ExitStack() as ph:
                    wop = ph.enter_context(tc.tile_pool(name="wo", bufs=1))
                    wo_t = wop.tile([128, QH, H], F32R)
                    nc.sync.dma_start(
                        wo_t[:], wo_d.rearrange("(cb p) n -> p cb n", p=128).bitcast(F32R))
                    psD = ph.enter_context(tc.tile_pool(name="psD", bufs=3, space="PSUM"))
                    outp = ph.enter_context(tc.tile_pool(name="outp", bufs=3))
                    for ob in range(H // 128):
                        for s2l in range(NSL):
                            po = psD.tile([128, SL], F32, tag="po")
                            for cb in range(QH):
                                nc.tensor.matmul(po[:], wo_t[:, cb, ob * 128:(ob + 1) * 128],
                                                 ctx[cb][:, s2l * SL:(s2l + 1) * SL],
                                                 start=(cb == 0), stop=(cb == QH - 1))
                            osb = outp.tile([128, SL], F32, tag="osb")
                            nc.scalar.copy(osb[:], po[:])
                            nc.sync.dma_start(
                                ap_d[ob * 128:(ob + 1) * 128, s2l * SL:(s2l + 1) * SL], osb[:])

    nc.compile()
    return nc


def build_l2():
    nc = bacc.Bacc("TRN2", target_bir_lowering=False, debug=False)

    hT_d = nc.dram_tensor("hT", [H, S], F32, kind="ExternalInput").ap()
    wg_d = nc.dram_tensor("wg", [H, FFS], F32, kind="ExternalInput").ap()
    wu_d = nc.dram_tensor("wu", [H, FFS], F32, kind="ExternalInput").ap()
    wd_d = nc.dram_tensor("wd", [FFS, H], F32, kind="ExternalInput").ap()
    wpost_d = nc.dram_tensor("wpost", [H], F32, kind="ExternalInput").ap()
    ones_d = nc.dram_tensor("ones", [128, 128], F32, kind="ExternalInput").ap()
    nit_d = nc.dram_tensor("nit", [1, 1], I32, kind="ExternalInput").ap()
    mlp_d = nc.dram_tensor("mlp_part", [S, H], F32, kind="ExternalOutput").ap()

    NFB = FFS // 128   # 6 ffn blocks per core

    with tile.TileContext(nc) as tc, ExitStack() as top:
        cpool = top.enter_context(tc.tile_pool(name="const0", bufs=1))
        nit_sb = cpool.tile([1, 1], I32)
        nc.sync.dma_start(nit_sb[:], nit_d[:])
        nv = _all_engine_value(nc, nit_sb[0:1, 0:1], "nit", 1 << 20)

        with tc.For_i(0, nv, 1, hint_engines=tuple(mybir.ALL_ENGINES)):
            with ExitStack() as it:
                const = it.enter_context(tc.tile_pool(name="c2", bufs=1))
                onesr = const.tile([128, 128], F32R)
                nc.sync.dma_start(onesr[:], ones_d[:].bitcast(F32R))
                wpost_sb = const.tile([128, NCH], F32)
                nc.sync.dma_start(wpost_sb[:], wpost_d.rearrange("(c p) -> p c", p=128))

                trow = it.enter_context(tc.tile_pool(name="trow", bufs=1))
                t_r = trow.tile([1, S], F32R, tag="t_r")
                tb = trow.tile([128, S], F32, tag="tb")

                actp = it.enter_context(tc.tile_pool(name="actp", bufs=1))
                act = actp.tile([128, NFB, S], F32R)

                # pass 1: hidden sum-of-squares -> t scales (before weights load)
                with ExitStack() as ph:
                    arow = ph.enter_context(tc.tile_pool(name="arow2", bufs=1))
                    ssq = arow.tile([1, S], F32, tag="ssq2")
                    rtmp = arow.tile([1, S], F32, tag="rtmp2")
                    hp0 = ph.enter_context(tc.tile_pool(name="ht0", bufs=4))
                    sqp = ph.enter_context(tc.tile_pool(name="sq0", bufs=3))
                    psS = ph.enter_context(tc.tile_pool(name="psS2", bufs=1, space="PSUM"))
                    psB = ph.enter_context(tc.tile_pool(name="psB2", bufs=2, space="PSUM"))
                    for ss in range(NSL):
                        ps_ss = psS.tile([1, SL], F32, tag="ssq_ps2")
                        for c in range(NCH):
                            ht = hp0.tile([128, SL], F32, tag="ht0")
                            nc.sync.dma_start(
                                ht[:], hT_d[c * 128:(c + 1) * 128, ss * SL:(ss + 1) * SL])
                            sq = sqp.tile([128, SL], F32R)
                            nc.scalar.activation(sq[:], ht[:], AF.Square)
                            nc.tensor.matmul(ps_ss[:], onesr[:, 0:1], sq[:],
                                             start=(c == 0), stop=(c == NCH - 1))
                        nc.scalar.copy(ssq[:, ss * SL:(ss + 1) * SL], ps_ss[:])
                    nc.vector.tensor_scalar(ssq[:], ssq[:], 1.0 / H, EPS,
                                            op0=ALU.mult, op1=ALU.add)
                    nc.vector.reciprocal(rtmp[:], ssq[:])
                    nc.scalar.sqrt(ssq[:], rtmp[:])
                    nc.vector.tensor_copy(t_r[:], ssq[:])
                    for sl4 in range(NSL):
                        pb = psB.tile([128, SL], F32, tag="tb_ps")
                        nc.tensor.matmul(pb[:], onesr[0:1, :], t_r[:, sl4 * SL:(sl4 + 1) * SL],
                                         start=True, stop=True)
                        nc.scalar.copy(tb[:, sl4 * SL:(sl4 + 1) * SL], pb[:])

                # gate/up projections -> silu(g*t) * (u*t), transposed layout
                with ExitStack() as ph:
                    wpool = ph.enter_context(tc.tile_pool(name="wgu", bufs=1))
                    wg_t = wpool.tile([128, NCH, FFS], F32R)
                    nc.sync.dma_start(
                        wg_t[:], wg_d.rearrange("(c p) n -> p c n", p=128).bitcast(F32R))
                    wu_t = wpool.tile([128, NCH, FFS], F32R)
                    nc.sync.dma_start(
                        wu_t[:], wu_d.rearrange("(c p) n -> p c n", p=128).bitcast(F32R))
                    for c in range(NCH):
                        sc = wpost_sb[:, c:c + 1]
                        nc.gpsimd.tensor_scalar(wg_t[:, c, :], wg_t[:, c, :].bitcast(F32),
                                                sc, None, op0=ALU.mult)
                        nc.gpsimd.tensor_scalar(wu_t[:, c, :], wu_t[:, c, :].bitcast(F32),
                                                sc, None, op0=ALU.mult)

                    hp = ph.enter_context(tc.tile_pool(name="ht2", bufs=4))
                    psA = ph.enter_context(tc.tile_pool(name="psA2", bufs=7, space="PSUM"))
                    silp = ph.enter_context(tc.tile_pool(name="silp", bufs=1))
                    gscp = ph.enter_context(tc.tile_pool(name="gscp", bufs=2))
                    aup = ph.enter_context(tc.tile_pool(name="aup", bufs=2))
                    for ss in range(NSL):
                        tsl = tb[:, ss * SL:(ss + 1) * SL]
                        gs_sil = []
                        for grp in range(2):
                            w_t = wg_t if grp == 0 else wu_t
                            ps = [psA.tile([128, SL], F32, tag="pgu", name=f"pgu{fb}")
                                  for fb in range(NFB)]
                            for c in range(NCH):
                                ht = hp.tile([128, SL], F32R, tag="ht2")
                                nc.sync.dma_start(
                                    ht[:], hT_d[c * 128:(c + 1) * 128,
                                                ss * SL:(ss + 1) * SL].bitcast(F32R))
                                for fb in range(NFB):
                                    nc.tensor.matmul(ps[fb][:], w_t[:, c, fb * 128:(fb + 1) * 128],
                                                     ht[:], start=(c == 0), stop=(c == NCH - 1))
                            if grp == 0:
                                for fb in range(NFB):
                                    gsct = gscp.tile([128, SL], F32, tag="gsc", name="gsc")
                                    nc.vector.tensor_tensor(gsct[:], ps[fb][:], tsl, op=ALU.mult)
                                    sig = gscp.tile([128, SL], F32, tag="sig", name="sig")
                                    nc.scalar.activation(sig[:], gsct[:], AF.Sigmoid)
                                    sil = silp.tile([128, SL], F32, tag=f"sil{fb}", name=f"sil{fb}")
                                    nc.vector.tensor_tensor(sil[:], gsct[:], sig[:], op=ALU.mult)
                                    gs_sil.append(sil)
                            else:
                                for fb in range(NFB):
                                    au = aup.tile([128, SL], F32, tag="au", name="au")
                                    nc.vector.tensor_tensor(au[:], ps[fb][:], tsl, op=ALU.mult)
                                    nc.vector.tensor_tensor(act[:, fb, ss * SL:(ss + 1) * SL],
                                                            au[:], gs_sil[fb][:], op=ALU.mult)

                # down projection -> natural [seq, out] partial
                with ExitStack() as ph:
                    wdp = ph.enter_context(tc.tile_pool(name="wd", bufs=1))
                    wd_t = wdp.tile([128, NFB, H], F32R)
                    nc.sync.dma_start(
                        wd_t[:], wd_d.rearrange("(f p) n -> p f n", p=128).bitcast(F32R))
                    psD = ph.enter_context(tc.tile_pool(name="psD2", bufs=3, space="PSUM"))
                    outp = ph.enter_context(tc.tile_pool(name="out2", bufs=3))
                    for sq16 in range(S // 128):
                        for os4 in range(NSL):
                            po = psD.tile([128, SL], F32, tag="po2")
                            for fb in range(NFB):
                                nc.tensor.matmul(po[:], act[:, fb, sq16 * 128:(sq16 + 1) * 128],
                                                 wd_t[:, fb, os4 * SL:(os4 + 1) * SL],
                                                 start=(fb == 0), stop=(fb == NFB - 1))
                            osb = outp.tile([128, SL], F32, tag="osb2")
                            nc.scalar.copy(osb[:], po[:])
                            nc.sync.dma_start(
                                mlp_d[sq16 * 128:(sq16 + 1) * 128, os4 * SL:(os4 + 1) * SL], osb[:])

    nc.compile()
    return nc


_L1 = None
_L2 = None


def _programs():
    global _L1, _L2
    if _L1 is None:
        _L1 = build_l1()
    if _L2 is None:
        _L2 = build_l2()
    return _L1, _L2


def _f32c(x):
    return np.ascontiguousarray(np.asarray(x), dtype=np.float32)


def kernel(hidden_states, causal_mask, keys_idxs, hs_idxs, positions,
           ln_in_w, q_w, k_w, v_w, q_norm_w, k_norm_w, o_w,
           ln_post_w, gate_w, up_w, down_w, _iters=1, _timing=None):
    l1, l2 = _programs()

    hidden = _f32c(hidden_states)[0]            # [S, H]
    keys = np.asarray(keys_idxs)
    hs = np.asarray(hs_idxs)
    pos = np.asarray(positions)[0]
    assert np.all(np.diff(np.asarray(keys, np.int64)) >= 0), "keys_idxs must be sorted"
    assert np.array_equal(keys, hs), "kernel assumes hs_idxs == keys_idxs (as in setup_inputs)"

    hT = np.ascontiguousarray(hidden.T)          # [H, S]
    q_w = _f32c(q_w); k_w = _f32c(k_w); v_w = _f32c(v_w); o_w = _f32c(o_w)
    ones128 = np.ones((128, 128), np.float32)
    pos_f = _f32c(pos).reshape(1, S)
    hs1_f = (_f32c(hs) + 1.0).reshape(1, S)
    keys_f = _f32c(keys)
    lnin_f = _f32c(ln_in_w)
    inv_freq = (1.0 / (1e6 ** (np.arange(0, HD, 2, dtype=np.float32) / HD))).astype(np.float32)
    invf_col = np.concatenate([inv_freq, inv_freq]).reshape(HD, 1).astype(np.float32)
    qnw_row = _f32c(q_norm_w).reshape(1, HD)
    knw_row = _f32c(k_norm_w).reshape(1, HD)
    nit = np.array([[_iters]], dtype=np.int32)

    in_maps = []
    for ci in range(NCORES):
        in_maps.append(dict(
            hT=hT,
            wq=np.ascontiguousarray(q_w[:, ci * QH * HD:(ci + 1) * QH * HD]),
            wk=np.ascontiguousarray(k_w[:, ci * HD:(ci + 1) * HD]),
            wv=np.ascontiguousarray(v_w[:, ci * HD:(ci + 1) * HD]),
            wo=np.ascontiguousarray(o_w[ci * QH * HD:(ci + 1) * QH * HD, :]),
            pos=pos_f, hs1=hs1_f, keys=keys_f, lnin=lnin_f,
            invf=invf_col, qnw=qnw_row, knw=knw_row, ones=ones128, nit=nit,
        ))
    t0 = time.time()
    r1 = run_bass_kernel_spmd(l1, in_maps, list(range(NCORES)))
    t1 = time.time()

    attn_T = np.zeros((H, S), np.float64)
    imp = np.zeros(S, np.float64)
    for ci in range(NCORES):
        attn_T += r1.results[ci]["attn_part"].astype(np.float64)
        imp += r1.results[ci]["imp_part"][0].astype(np.float64)
    imp = (imp / NH).astype(np.float32)
    imp[S - 1] = np.inf

    hT2 = np.ascontiguousarray((hT.astype(np.float64) + attn_T).astype(np.float32))

    gate_w = _f32c(gate_w); up_w = _f32c(up_w); down_w = _f32c(down_w)
    wpost_f = _f32c(ln_post_w)
    in_maps2 = []
    for ci in range(NCORES):
        in_maps2.append(dict(
            hT=hT2,
            wg=np.ascontiguousarray(gate_w[:, ci * FFS:(ci + 1) * FFS]),
            wu=np.ascontiguousarray(up_w[:, ci * FFS:(ci + 1) * FFS]),
            wd=np.ascontiguousarray(down_w[ci * FFS:(ci + 1) * FFS, :]),
            wpost=wpost_f, ones=ones128, nit=nit,
        ))
    t2 = time.time()
    r2 = run_bass_kernel_spmd(l2, in_maps2, list(range(NCORES)))
    t3 = time.time()

    mlp = np.zeros((S, H), np.float64)
    for ci in range(NCORES):
        mlp += r2.results[ci]["mlp_part"].astype(np.float64)
    out = (hT2.T.astype(np.float64) + mlp).astype(np.float32)[None]

    if _timing is not None:
        _timing["l1_wall"] = t1 - t0
        _timing["l2_wall"] = t3 - t2
    return out, imp
